# revision 31
# baseline (speedup 1.0000x reference)
"""MoE routing gate kernel for Trainium2 (8 NeuronCores, data-parallel).

Computes, for x[32768, 2048], weight[64, 2048], bias[64]:
    logits = x @ weight.T
    probs  = softmax(logits, axis=-1)
    idx    = top_k(probs + bias, 6).indices
    w      = take_along_axis(probs, idx)
returning (w float32 [32768, 6], idx int32 [32768, 6]).

Sharding: tokens split 4096/core across 8 cores; weight/bias replicated.

Per-core pipeline (memory-bound; the knob is bytes-of-x per element):
  - x ships as fp16 hi (2B) + e4m3 lo of (x - hi)*2^12 (1B) = 3B/elem,
    25% less HBM traffic than fp32/bf16-hi-lo.  Three bf16/mixed passes
    accumulate fp32 in PSUM: x_hi@w_hi + x_hi@w_lo + x_lo@(w*2^-12),
    with w_hi/w_lo a bf16 hi/lo split of the fp32 weights.  Logit rms
    error ~5.5e-6 (same as an all-fp32 pipeline's top-k stability).
  - 512-token supergroups keep the post-matmul (DVE-bound) stream fed
    from ~12us in and shrink the pipeline tail to one small supergroup.
  - Matmul pairs are column-tiled: group g=0 lands in PE columns 0-63
    (PSUM partitions 0-63), g=1 in columns 64-127, so two N=256 matmuls
    stream concurrently through disjoint column groups of the array.
    Both hi passes sweep k first (needs only the two hi DMAs), the lo
    sweep accumulates last (lo DMA has the whole hi sweep to land).
  - logits^T [64,256] -> ACT copy to SBUF -> PE transpose (identity
    matmul) back to [128 tokens, 64 experts] tiles in PSUM.
  - Softmax without max-subtraction (|logits| < ~7): ACT exp reads PSUM
    and emits the row sum via accum_out.  Selection key
    q = exp + sum*bias ranks identically to probs + bias (fp32).
  - DVE Max8/MaxIndex8 give top-8 values+indices; the top-6 unbiased
    weights come from 6 (iota == idx_k) * exp gathers with accum_out in
    bf16 (weights only need ~1e-3).  ACT casts the indices / exp to
    bf16 and applies the final 1/sum scale (Copy with per-partition
    scale), keeping DVE (the serial bottleneck engine) op count low.
  - A ~4us junk-matmul warmup opens the PE HAM clock gate before the
    first real accumulation sweep.
  - Consts ride in 2 packed DMAs on the scalar ring; outputs go out per
    supergroup on the otherwise-idle GpSimd (SWDGE) ring.
"""

import numpy as np
import ml_dtypes

import concourse.bacc as bacc
import concourse.bass as bass
import concourse.mybir as mybir
import concourse.tile as tile
from concourse.bass_utils import run_bass_kernel_spmd

BF16 = mybir.dt.bfloat16
F16 = mybir.dt.float16
F8E4 = mybir.dt.float8e4
F32 = mybir.dt.float32
I32 = mybir.dt.int32
U32 = mybir.dt.uint32
OP = mybir.AluOpType
EXP = mybir.ActivationFunctionType.Exp
COPY = mybir.ActivationFunctionType.Copy

TOKENS, DIM, E, TOPK, NCORES = 32768, 2048, 64, 6, 8
KC = DIM // 128      # contraction chunks of 128
KH = KC // 2         # k-chunks per hi DMA
LOSCALE = 4096.0     # 2^12: residual scale for the e4m3 lo stream
SGT = 512            # tokens per supergroup


def build_nc(tpc, sg_t=SGT):
    """Build the per-core Bass program for a tpc-token shard."""
    grp = sg_t // 2         # tokens per matmul (N), two col-tiled groups per sg
    nsg = tpc // sg_t
    nj = sg_t // 128        # 128-token tiles per super-group

    nc = bacc.Bacc("TRN2", target_bir_lowering=False, debug=False)

    xhi = nc.dram_tensor("xhi", [nsg, 2, 128, KH, sg_t], F16, kind="ExternalInput")
    xlo = nc.dram_tensor("xlo", [nsg, 128, KC, sg_t], F8E4, kind="ExternalInput")
    wpack = nc.dram_tensor("wpack", [128, 3, KC, E], BF16, kind="ExternalInput")
    fpack = nc.dram_tensor("fpack", [128, 3, E], F32, kind="ExternalInput")
    w_out = nc.dram_tensor("w_out", [nsg, 128, nj, TOPK], F32, kind="ExternalOutput")
    i_out = nc.dram_tensor("i_out", [nsg, 128, nj, TOPK], I32, kind="ExternalOutput")

    with tile.TileContext(nc) as tc:
        with (
            tc.tile_pool(name="consts", bufs=1) as cpool,
            tc.tile_pool(name="xh", bufs=8) as xhp,
            tc.tile_pool(name="xl", bufs=4) as xlp,
            tc.tile_pool(name="lt", bufs=3) as ltp,
            tc.tile_pool(name="slab", bufs=3) as slp,
            tc.tile_pool(name="work", bufs=4) as wkp,
            tc.tile_pool(name="stage", bufs=3) as stp,
            tc.tile_pool(name="acc", bufs=3, space="PSUM") as accp,
            tc.tile_pool(name="tr", bufs=4, space="PSUM") as trp,
        ):
            cw = cpool.tile([128, 3, KC, E], BF16)
            nc.scalar.dma_start(cw, wpack[:])
            cf = cpool.tile([128, 3, E], F32)
            nc.scalar.dma_start(cf, fpack[:])
            cbias = cf[:, 0]
            ciota = cf[:, 1]
            cident = cf[:, 2]

            # PE warmup: ~4us of junk matmuls so the HAM clock-gate opens
            # before the first real accumulation sweep arrives (saves the
            # 1.2GHz cold stretch on supergroup 0).
            wmt = cpool.tile([128, 128], F32)
            nc.vector.memset(wmt, 0.0)
            wps = trp.tile([128, 128], F32, tag="warm", bufs=1)
            for i in range(36):
                nc.tensor.matmul(
                    wps, wmt[:, 0:128], wmt[:, 0:128],
                    start=(i == 0), stop=(i == 35),
                )

            # Gathers are deferred by one tile in the DVE queue: the held
            # work drains during the acc-close -> transpose -> exp latency
            # at each supergroup boundary instead of DVE idling there.
            gq = []

            def emit_gather(ex, idxb, sw_j, rs_j, fin):
                g6 = wkp.tile([128, TOPK], F32, tag="g6")
                for kk in range(TOPK):
                    scr = wkp.tile([128, E], F32, tag="scrv", bufs=3)
                    nc.vector.scalar_tensor_tensor(
                        scr, ciota, idxb[:, kk:kk + 1], ex,
                        OP.is_equal, OP.mult,
                        accum_out=g6[:, kk:kk + 1],
                    )
                nc.scalar.activation(sw_j, g6, COPY, scale=rs_j)
                if fin is not None:
                    si_t, mis_t, sg_i, sw_t = fin
                    nc.vector.tensor_copy(si_t, mis_t[:, :, 0:TOPK])
                    nc.gpsimd.dma_start(w_out[sg_i], sw_t)
                    nc.gpsimd.dma_start(i_out[sg_i], si_t)

            for sg in range(nsg):
                xh0 = xhp.tile([128, KH, sg_t], F16, tag="xh")
                nc.sync.dma_start(xh0, xhi[sg, 0])
                xh1 = xhp.tile([128, KH, sg_t], F16, tag="xh")
                nc.sync.dma_start(xh1, xhi[sg, 1])
                xl = xlp.tile([128, KC, sg_t], F8E4, tag="xl")
                nc.sync.dma_start(xl, xlo[sg])
                xh = (xh0, xh1)

                sw = stp.tile([128, nj, TOPK], F32, tag="sw")
                si = stp.tile([128, nj, TOPK], I32, tag="si")
                ssum = slp.tile([128, nj], F32, tag="ssum")
                mis = slp.tile([128, nj, 8], U32, tag="mis")
                rs = slp.tile([128, nj], F32, tag="rs")

                # sg0's compute is split into two 256-token halves so its
                # first logits close ~1.3us after the lo DMA lands instead
                # of after a full-width sweep: the DVE stream starts ~5us
                # earlier (token mapping j*128+p is unchanged).
                halves = [(0, sg_t)] if sg else [(0, sg_t // 2),
                                                 (sg_t // 2, sg_t // 2)]
                for toff, tlen in halves:
                    grp_l = tlen // 2
                    nj_l = tlen // 128
                    jb = toff // 128
                    acc = accp.tile([128, grp], F32)
                    # hi sweep: per k, w_hi and w_lo against fp16 x
                    for k in range(KC):
                        xk = xh[k // KH][:, k % KH]  # [128, sg_t] f16
                        for p in range(2):
                            wv = cw[:, p, k]
                            first = (k == 0 and p == 0)
                            nc.tensor.matmul(
                                acc[0:64, 0:grp_l], wv,
                                xk[:, toff:toff + grp_l],
                                start=first, stop=False, tile_position=(0, 0),
                            )
                            nc.tensor.matmul(
                                acc[64:128, 0:grp_l], wv,
                                xk[:, toff + grp_l:toff + tlen],
                                start=first, stop=False, tile_position=(0, 64),
                                skip_group_check=True,
                            )
                    # lo sweep: scaled residual against w*2^-12
                    for k in range(KC):
                        xk = xl[:, k]
                        last = (k == KC - 1)
                        nc.tensor.matmul(
                            acc[0:64, 0:grp_l], cw[:, 2, k],
                            xk[:, toff:toff + grp_l],
                            start=False, stop=last, tile_position=(0, 0),
                        )
                        nc.tensor.matmul(
                            acc[64:128, 0:grp_l], cw[:, 2, k],
                            xk[:, toff + grp_l:toff + tlen],
                            start=False, stop=last, tile_position=(0, 64),
                            skip_group_check=True,
                        )

                    lt = ltp.tile([128, grp], F32, tag="lt")
                    nc.scalar.copy(lt[:, 0:grp_l], acc[:, 0:grp_l])

                    for jl in range(nj_l):
                        j = jb + jl
                        base = 64 * (jl // (nj_l // 2))
                        jj = jl % (nj_l // 2)
                        tps = trp.tile([128, E], F32)
                        nc.tensor.transpose(
                            tps,
                            lt[base:base + 64, jj * 128:(jj + 1) * 128],
                            cident[base:base + 64, :],
                        )
                        ex = wkp.tile([128, E], F32, tag="ex", bufs=nj + 2)
                        nc.scalar.activation(
                            ex, tps, EXP, accum_out=ssum[:, j:j + 1]
                        )
                        q = wkp.tile([128, E], F32, tag="q")
                        nc.vector.scalar_tensor_tensor(
                            q, cbias, ssum[:, j:j + 1], ex, OP.mult, OP.add
                        )
                        mx = wkp.tile([128, 8], F32, tag="mx")
                        nc.vector.max(mx, q)
                        nc.vector.max_index(mis[:, j], mx, q)
                        nc.vector.reciprocal(rs[:, j:j + 1], ssum[:, j:j + 1])
                        fin = (si, mis, sg, sw) if j == nj - 1 else None
                        gq.append(
                            (ex, mis[:, j], sw[:, j], rs[:, j:j + 1], fin)
                        )
                        if len(gq) > 1:
                            emit_gather(*gq.pop(0))
            while gq:
                emit_gather(*gq.pop(0))
    return nc


_CACHE = {}


def _get_compiled(tpc):
    if tpc not in _CACHE:
        nc = build_nc(tpc)
        nc.compile()
        _CACHE[tpc] = nc
    return _CACHE[tpc]


def _prep_shared(weight, bias):
    bf = ml_dtypes.bfloat16
    w = np.asarray(weight, np.float32)
    w_hi = w.astype(bf).astype(np.float32)
    w_lo = (w - w_hi).astype(bf)
    w_s = (w * (1.0 / LOSCALE)).astype(bf)

    def wtile(a):  # [E, DIM] -> [128, KC, E]
        return np.ascontiguousarray(
            np.ascontiguousarray(a.T).reshape(KC, 128, E).transpose(1, 0, 2)
        )

    wpack = np.stack(
        [wtile(w_hi.astype(bf)), wtile(w_lo), wtile(w_s)], axis=1
    )  # [128, 3, KC, E]

    fpack = np.stack(
        [
            np.broadcast_to(np.asarray(bias, np.float32), (128, E)),
            np.broadcast_to(np.arange(E, dtype=np.float32), (128, E)),
            np.tile(np.eye(64, dtype=np.float32), (2, 1)),
        ],
        axis=1,
    )  # [128, 3, E]

    return {
        "wpack": np.ascontiguousarray(wpack),
        "fpack": np.ascontiguousarray(fpack),
    }


def prep_core_inputs(x, weight, bias, ncores=NCORES, sg_t=SGT):
    f8 = ml_dtypes.float8_e4m3
    shared = _prep_shared(weight, bias)
    x = np.asarray(x, np.float32)
    tpc = x.shape[0] // ncores
    nsg = tpc // sg_t
    in_maps = []
    for c in range(ncores):
        xs = np.ascontiguousarray(x[c * tpc:(c + 1) * tpc].T)  # [DIM, tpc]
        xh = xs.astype(np.float16)
        xl = ((xs - xh.astype(np.float32)) * LOSCALE).astype(f8)
        # hi: [nsg, 2, 128, KH, sg_t]; per (sg, half, partition) one 8KB run
        h6 = xh.reshape(2, KH, 128, nsg, sg_t)
        pk_h = np.ascontiguousarray(h6.transpose(3, 0, 2, 1, 4))
        # lo: [nsg, 128, KC, sg_t]; per (sg, partition) one 8KB run
        l6 = xl.reshape(KC, 128, nsg, sg_t)
        pk_l = np.ascontiguousarray(l6.transpose(2, 1, 0, 3))
        in_maps.append({"xhi": pk_h, "xlo": pk_l, **shared})
    return in_maps


def unpack_outputs(res_list, tpc):
    ws, idxs = [], []
    for r in res_list:
        wv = np.asarray(r["w_out"])  # [nsg, 128, nj, TOPK]
        iv = np.asarray(r["i_out"])
        nsg = wv.shape[0]
        wv = wv.reshape(nsg, 128, -1, TOPK).transpose(0, 2, 1, 3).reshape(tpc, TOPK)
        iv = iv.reshape(nsg, 128, -1, TOPK).transpose(0, 2, 1, 3).reshape(tpc, TOPK)
        ws.append(wv)
        idxs.append(iv)
    return (
        np.ascontiguousarray(np.concatenate(ws)).astype(np.float32),
        np.ascontiguousarray(np.concatenate(idxs)).astype(np.int32),
    )


def run(x, weight, bias, trace=False, **kwargs):
    x = np.asarray(x, np.float32)
    tpc = x.shape[0] // NCORES
    nc = _get_compiled(tpc)
    in_maps = prep_core_inputs(x, weight, bias)
    res = run_bass_kernel_spmd(nc, in_maps, list(range(NCORES)), trace=trace, **kwargs)
    w, i = unpack_outputs(res.results, tpc)
    return w, i, res


def kernel(x, weight, bias):
    w, i, _ = run(x, weight, bias, trace=False)
    return w, i


# revision 32
# speedup vs baseline: 1.1696x; 1.1696x over previous
"""MoE routing gate kernel for Trainium2 (8 NeuronCores, data-parallel).

Computes, for x[32768, 2048], weight[64, 2048], bias[64]:
    logits = x @ weight.T
    probs  = softmax(logits, axis=-1)
    idx    = top_k(probs + bias, 6).indices
    w      = take_along_axis(probs, idx)
returning (w float32 [32768, 6], idx int32 [32768, 6]).

Sharding: tokens split 4096/core across 8 cores; weight/bias replicated.

Per-core pipeline (memory-bound; the knob is bytes-of-x per element):
  - x ships as fp16 hi (2B) + e4m3 lo of (x - hi)*2^12 (1B) = 3B/elem,
    25% less HBM traffic than fp32/bf16-hi-lo.  Three bf16/mixed passes
    accumulate fp32 in PSUM: x_hi@w_hi + x_hi@w_lo + x_lo@(w*2^-12),
    with w_hi/w_lo a bf16 hi/lo split of the fp32 weights.  Logit rms
    error ~5.5e-6 (same as an all-fp32 pipeline's top-k stability).
  - 512-token supergroups keep the post-matmul (DVE-bound) stream fed
    from ~12us in and shrink the pipeline tail to one small supergroup.
  - Matmul pairs are column-tiled: group g=0 lands in PE columns 0-63
    (PSUM partitions 0-63), g=1 in columns 64-127, so two N=256 matmuls
    stream concurrently through disjoint column groups of the array.
    Both hi passes sweep k first (needs only the two hi DMAs), the lo
    sweep accumulates last (lo DMA has the whole hi sweep to land).
  - logits^T [64,256] -> ACT copy to SBUF -> PE transpose (identity
    matmul) back to [128 tokens, 64 experts] tiles in PSUM.
  - Softmax without max-subtraction (|logits| < ~7): ACT exp reads PSUM
    and emits the row sum via accum_out.  Selection key
    q = exp + sum*bias ranks identically to probs + bias (fp32).
  - DVE Max8/MaxIndex8 give top-8 values+indices; the top-6 unbiased
    weights come from 6 (iota == idx_k) * exp gathers with accum_out in
    bf16 (weights only need ~1e-3).  ACT casts the indices / exp to
    bf16 and applies the final 1/sum scale (Copy with per-partition
    scale), keeping DVE (the serial bottleneck engine) op count low.
  - A ~4us junk-matmul warmup opens the PE HAM clock gate before the
    first real accumulation sweep.
  - Consts ride in 2 packed DMAs on the scalar ring; outputs go out per
    supergroup on the otherwise-idle GpSimd (SWDGE) ring.
"""

import numpy as np
import ml_dtypes

import concourse.bacc as bacc
import concourse.bass as bass
import concourse.mybir as mybir
import concourse.tile as tile
from concourse.bass_utils import run_bass_kernel_spmd

BF16 = mybir.dt.bfloat16
F16 = mybir.dt.float16
F8E4 = mybir.dt.float8e4
F32 = mybir.dt.float32
I32 = mybir.dt.int32
U32 = mybir.dt.uint32
OP = mybir.AluOpType
EXP = mybir.ActivationFunctionType.Exp
COPY = mybir.ActivationFunctionType.Copy

TOKENS, DIM, E, TOPK, NCORES = 32768, 2048, 64, 6, 8
KC = DIM // 128      # contraction chunks of 128
KH = KC // 2         # k-chunks per hi DMA
LOSCALE = 4096.0     # 2^12: residual scale for the e4m3 lo stream
SGT = 512            # tokens per supergroup


def build_nc(tpc, sg_t=SGT):
    """Build the per-core Bass program for a tpc-token shard."""
    grp = sg_t // 2         # tokens per matmul (N), two col-tiled groups per sg
    nsg = tpc // sg_t
    nj = sg_t // 128        # 128-token tiles per super-group

    nc = bacc.Bacc("TRN2", target_bir_lowering=False, debug=False)

    xhi = nc.dram_tensor("xhi", [nsg, 2, 128, KH, sg_t], F16, kind="ExternalInput")
    xlo = nc.dram_tensor("xlo", [nsg, 128, KC, sg_t], F8E4, kind="ExternalInput")
    wpack = nc.dram_tensor("wpack", [128, 3, KC, E], BF16, kind="ExternalInput")
    fpack = nc.dram_tensor("fpack", [128, 3, E], F32, kind="ExternalInput")
    w_out = nc.dram_tensor("w_out", [nsg, 128, nj, TOPK], F32, kind="ExternalOutput")
    i_out = nc.dram_tensor("i_out", [nsg, 128, nj, TOPK], I32, kind="ExternalOutput")

    with tile.TileContext(nc) as tc:
        with (
            tc.tile_pool(name="consts", bufs=1) as cpool,
            tc.tile_pool(name="xh", bufs=8) as xhp,
            tc.tile_pool(name="xl", bufs=4) as xlp,
            tc.tile_pool(name="lt", bufs=3) as ltp,
            tc.tile_pool(name="slab", bufs=3) as slp,
            tc.tile_pool(name="work", bufs=4) as wkp,
            tc.tile_pool(name="stage", bufs=3) as stp,
            tc.tile_pool(name="acc", bufs=3, space="PSUM") as accp,
            tc.tile_pool(name="tr", bufs=4, space="PSUM") as trp,
        ):
            cw = cpool.tile([128, 3, KC, E], BF16)
            nc.scalar.dma_start(cw, wpack[:])
            cf = cpool.tile([128, 3, E], F32)
            nc.scalar.dma_start(cf, fpack[:])
            cbias = cf[:, 0]
            ciota = cf[:, 1]
            cident = cf[:, 2]

            # PE warmup: ~4us of junk matmuls so the HAM clock-gate opens
            # before the first real accumulation sweep arrives (saves the
            # 1.2GHz cold stretch on supergroup 0).
            wmt = cpool.tile([128, 128], F32)
            nc.vector.memset(wmt, 0.0)
            wps = trp.tile([128, 128], F32, tag="warm", bufs=1)
            for i in range(36):
                nc.tensor.matmul(
                    wps, wmt[:, 0:128], wmt[:, 0:128],
                    start=(i == 0), stop=(i == 35),
                )

            # Gathers are deferred by one tile in the DVE queue: the held
            # work drains during the acc-close -> transpose -> exp latency
            # at each supergroup boundary instead of DVE idling there.
            gq = []

            def emit_gather(ex, idxb, sw_j, rs_j, fin):
                g6 = wkp.tile([128, TOPK], F32, tag="g6")
                for kk in range(TOPK):
                    scr = wkp.tile([128, E], F32, tag="scrv", bufs=3)
                    nc.vector.scalar_tensor_tensor(
                        scr, ciota, idxb[:, kk:kk + 1], ex,
                        OP.is_equal, OP.mult,
                        accum_out=g6[:, kk:kk + 1],
                    )
                nc.scalar.activation(sw_j, g6, COPY, scale=rs_j)
                if fin is not None:
                    si_t, mis_t, sg_i, sw_t = fin
                    nc.vector.tensor_copy(si_t, mis_t[:, :, 0:TOPK])
                    nc.gpsimd.dma_start(w_out[sg_i], sw_t)
                    nc.gpsimd.dma_start(i_out[sg_i], si_t)

            for sg in range(nsg):
                xh0 = xhp.tile([128, KH, sg_t], F16, tag="xh")
                nc.sync.dma_start(xh0, xhi[sg, 0])
                xh1 = xhp.tile([128, KH, sg_t], F16, tag="xh")
                nc.sync.dma_start(xh1, xhi[sg, 1])
                xl = xlp.tile([128, KC, sg_t], F8E4, tag="xl")
                nc.sync.dma_start(xl, xlo[sg])
                xh = (xh0, xh1)

                sw = stp.tile([128, nj, TOPK], F32, tag="sw")
                si = stp.tile([128, nj, TOPK], I32, tag="si")
                ssum = slp.tile([128, nj], F32, tag="ssum")
                mis = slp.tile([128, nj, 8], U32, tag="mis")
                rs = slp.tile([128, nj], F32, tag="rs")

                # sg0's compute is split into two 256-token halves so its
                # first logits close ~1.3us after the lo DMA lands instead
                # of after a full-width sweep: the DVE stream starts ~5us
                # earlier (token mapping j*128+p is unchanged).
                halves = [(0, sg_t)] if sg else [(0, sg_t // 2),
                                                 (sg_t // 2, sg_t // 2)]
                for toff, tlen in halves:
                    grp_l = tlen // 2
                    nj_l = tlen // 128
                    jb = toff // 128
                    acc = accp.tile([128, grp], F32)
                    # hi sweep: per k, w_hi and w_lo against fp16 x
                    for k in range(KC):
                        xk = xh[k // KH][:, k % KH]  # [128, sg_t] f16
                        for p in range(2):
                            wv = cw[:, p, k]
                            first = (k == 0 and p == 0)
                            nc.tensor.matmul(
                                acc[0:64, 0:grp_l], wv,
                                xk[:, toff:toff + grp_l],
                                start=first, stop=False, tile_position=(0, 0),
                            )
                            nc.tensor.matmul(
                                acc[64:128, 0:grp_l], wv,
                                xk[:, toff + grp_l:toff + tlen],
                                start=first, stop=False, tile_position=(0, 64),
                                skip_group_check=True,
                            )
                    # lo sweep: scaled residual against w*2^-12
                    for k in range(KC):
                        xk = xl[:, k]
                        last = (k == KC - 1)
                        nc.tensor.matmul(
                            acc[0:64, 0:grp_l], cw[:, 2, k],
                            xk[:, toff:toff + grp_l],
                            start=False, stop=last, tile_position=(0, 0),
                        )
                        nc.tensor.matmul(
                            acc[64:128, 0:grp_l], cw[:, 2, k],
                            xk[:, toff + grp_l:toff + tlen],
                            start=False, stop=last, tile_position=(0, 64),
                            skip_group_check=True,
                        )

                    lt = ltp.tile([128, grp], F32, tag="lt")
                    nc.scalar.copy(lt[:, 0:grp_l], acc[:, 0:grp_l])

                    for jl in range(nj_l):
                        j = jb + jl
                        base = 64 * (jl // (nj_l // 2))
                        jj = jl % (nj_l // 2)
                        tps = trp.tile([128, E], F32)
                        nc.tensor.transpose(
                            tps,
                            lt[base:base + 64, jj * 128:(jj + 1) * 128],
                            cident[base:base + 64, :],
                        )
                        ex = wkp.tile([128, E], F32, tag="ex", bufs=nj + 2)
                        nc.scalar.activation(
                            ex, tps, EXP, accum_out=ssum[:, j:j + 1]
                        )
                        q = wkp.tile([128, E], F32, tag="q")
                        nc.vector.scalar_tensor_tensor(
                            q, cbias, ssum[:, j:j + 1], ex, OP.mult, OP.add
                        )
                        mx = wkp.tile([128, 8], F32, tag="mx")
                        nc.vector.max(mx, q)
                        nc.vector.max_index(mis[:, j], mx, q)
                        idxb = wkp.tile([128, 8], F32, tag="idxb", bufs=6)
                        nc.scalar.copy(idxb, mis[:, j])
                        nc.vector.reciprocal(rs[:, j:j + 1], ssum[:, j:j + 1])
                        fin = (si, mis, sg, sw) if j == nj - 1 else None
                        gq.append(
                            (ex, idxb, sw[:, j], rs[:, j:j + 1], fin)
                        )
                        if len(gq) > 1:
                            emit_gather(*gq.pop(0))
            while gq:
                emit_gather(*gq.pop(0))
    return nc


_CACHE = {}


def _get_compiled(tpc):
    if tpc not in _CACHE:
        nc = build_nc(tpc)
        nc.compile()
        _CACHE[tpc] = nc
    return _CACHE[tpc]


def _prep_shared(weight, bias):
    bf = ml_dtypes.bfloat16
    w = np.asarray(weight, np.float32)
    w_hi = w.astype(bf).astype(np.float32)
    w_lo = (w - w_hi).astype(bf)
    w_s = (w * (1.0 / LOSCALE)).astype(bf)

    def wtile(a):  # [E, DIM] -> [128, KC, E]
        return np.ascontiguousarray(
            np.ascontiguousarray(a.T).reshape(KC, 128, E).transpose(1, 0, 2)
        )

    wpack = np.stack(
        [wtile(w_hi.astype(bf)), wtile(w_lo), wtile(w_s)], axis=1
    )  # [128, 3, KC, E]

    fpack = np.stack(
        [
            np.broadcast_to(np.asarray(bias, np.float32), (128, E)),
            np.broadcast_to(np.arange(E, dtype=np.float32), (128, E)),
            np.tile(np.eye(64, dtype=np.float32), (2, 1)),
        ],
        axis=1,
    )  # [128, 3, E]

    return {
        "wpack": np.ascontiguousarray(wpack),
        "fpack": np.ascontiguousarray(fpack),
    }


def prep_core_inputs(x, weight, bias, ncores=NCORES, sg_t=SGT):
    f8 = ml_dtypes.float8_e4m3
    shared = _prep_shared(weight, bias)
    x = np.asarray(x, np.float32)
    tpc = x.shape[0] // ncores
    nsg = tpc // sg_t
    in_maps = []
    for c in range(ncores):
        xs = np.ascontiguousarray(x[c * tpc:(c + 1) * tpc].T)  # [DIM, tpc]
        xh = xs.astype(np.float16)
        xl = ((xs - xh.astype(np.float32)) * LOSCALE).astype(f8)
        # hi: [nsg, 2, 128, KH, sg_t]; per (sg, half, partition) one 8KB run
        h6 = xh.reshape(2, KH, 128, nsg, sg_t)
        pk_h = np.ascontiguousarray(h6.transpose(3, 0, 2, 1, 4))
        # lo: [nsg, 128, KC, sg_t]; per (sg, partition) one 8KB run
        l6 = xl.reshape(KC, 128, nsg, sg_t)
        pk_l = np.ascontiguousarray(l6.transpose(2, 1, 0, 3))
        in_maps.append({"xhi": pk_h, "xlo": pk_l, **shared})
    return in_maps


def unpack_outputs(res_list, tpc):
    ws, idxs = [], []
    for r in res_list:
        wv = np.asarray(r["w_out"])  # [nsg, 128, nj, TOPK]
        iv = np.asarray(r["i_out"])
        nsg = wv.shape[0]
        wv = wv.reshape(nsg, 128, -1, TOPK).transpose(0, 2, 1, 3).reshape(tpc, TOPK)
        iv = iv.reshape(nsg, 128, -1, TOPK).transpose(0, 2, 1, 3).reshape(tpc, TOPK)
        ws.append(wv)
        idxs.append(iv)
    return (
        np.ascontiguousarray(np.concatenate(ws)).astype(np.float32),
        np.ascontiguousarray(np.concatenate(idxs)).astype(np.int32),
    )


def run(x, weight, bias, trace=False, **kwargs):
    x = np.asarray(x, np.float32)
    tpc = x.shape[0] // NCORES
    nc = _get_compiled(tpc)
    in_maps = prep_core_inputs(x, weight, bias)
    res = run_bass_kernel_spmd(nc, in_maps, list(range(NCORES)), trace=trace, **kwargs)
    w, i = unpack_outputs(res.results, tpc)
    return w, i, res


def kernel(x, weight, bias):
    w, i, _ = run(x, weight, bias, trace=False)
    return w, i
